# revision 55
# baseline (speedup 1.0000x reference)
"""MultiHeadAttention TRN2 kernel: tensor-parallel over heads across 8 NeuronCores.

Problem (hardcoded): BS=2, QLEN=2048, DIM=1024, NHEADS=16, HEAD=64.
  q = split_heads(x @ q_w.T + q_b) / sqrt(64)
  s = q @ k.T + mask ; w = softmax(s) ; ctx = w @ v
  out = merge_heads(ctx) @ o_w.T + o_b

Sharding: core c computes heads {2c, 2c+1} (rows 128c:128c+128 of q/k/v weights,
cols 128c:128c+128 of o_w).  Each core emits a full-shape bf16 partial of the
output projection; the host sums the 8 partials in f32 and adds o_b.

v2 design notes (vs the v1 baseline at 475us):
- All DRAM operands are pre-arranged on the host so every dma_start moves big
  contiguous rows (the SWDGE trigger costs ~10ns/descriptor on the issuing
  engine queue); DMA triggers are spread across sync/gpsimd/vector/scalar.
- V is transposed on the PE (transpose-mode matmul, f32 PSUM out) instead of
  descriptor-exploding DMA transposes.
- V stationary tiles carry an extra all-ones column, so each PV matmul also
  produces the softmax denominator in PSUM partition 64 for free -> the 25% of
  matmuls the baseline spent on ones-matmul row sums are gone.
- Attention is ACT-paced: exp of scores is the hard floor (~16.8M elems/core
  through the scalar engine).  The qtile loop is software-pipelined 3 deep
  (scores+exp for qtile i, PV for i-1, normalize+out-proj for i-2) with a
  double-buffered st so the PE/DVE work hides under the exp stream.
- exp is fused over both heads of one k-tile ([128, 2x512] PSUM AP, mask fed
  through the activation bias port).
"""

import sys

if "/opt/trn_rl_repo" not in sys.path:
    sys.path.insert(0, "/opt/trn_rl_repo")

import math
from contextlib import ExitStack

import ml_dtypes
import numpy as np

import concourse.bass as bass
import concourse.tile as tile
from concourse import bacc, masks, mybir
from concourse.bass_utils import run_bass_kernel_spmd


# ---- problem constants ----
BS, QLEN, DIM, NHEADS = 2, 2048, 1024, 16
HEAD = DIM // NHEADS            # 64
NTOK = BS * QLEN                # 4096
NCORES = 8
HPC = NHEADS // NCORES          # 2 heads per core
LDIM = HPC * HEAD               # 128 local dims per core
NKCH = DIM // 128               # 8 contraction chunks for projections
NTT = NTOK // 512               # 8 token groups of 512 for projections
NKT = QLEN // 128               # 16 key tiles per batch
QTW = 512                       # query tile width for attention
NQT = NTOK // QTW               # 8 query tiles total (4 per batch)

DT = mybir.dt.bfloat16          # matmul compute dtype
NPDT = ml_dtypes.bfloat16
F32 = mybir.dt.float32

_cache = {}


def build_program():
    """Build + compile the single-core SPMD Bass program."""
    nc = bacc.Bacc("TRN2", target_bir_lowering=False, debug=False,
                   num_devices=NCORES)

    xt = nc.dram_tensor("xt", [128, NKCH, NTOK], DT, kind="ExternalInput").ap()
    wq = nc.dram_tensor("wq", [128, NKCH, LDIM], DT, kind="ExternalInput").ap()
    wk = nc.dram_tensor("wk", [128, NKCH, LDIM], DT, kind="ExternalInput").ap()
    wv = nc.dram_tensor("wv", [128, NKCH, LDIM], DT, kind="ExternalInput").ap()
    wo = nc.dram_tensor("wo", [LDIM, DIM], DT, kind="ExternalInput").ap()
    qb = nc.dram_tensor("qb", [LDIM, 1], F32, kind="ExternalInput").ap()
    kb = nc.dram_tensor("kb", [LDIM, 1], F32, kind="ExternalInput").ap()
    vb = nc.dram_tensor("vb", [LDIM, 1], F32, kind="ExternalInput").ap()
    maskd = nc.dram_tensor("maskd", [128, BS * NKT], F32,
                           kind="ExternalInput").ap()
    out = nc.dram_tensor("out", [NTOK, DIM], DT, kind="ExternalOutput").ap()

    with tile.TileContext(nc) as tc, ExitStack() as ctx:
        singles = ctx.enter_context(tc.tile_pool(name="singles", bufs=1))
        xpool = ctx.enter_context(tc.tile_pool(name="xs", bufs=12))
        vtp = ctx.enter_context(tc.tile_pool(name="vt", bufs=8))
        stp = ctx.enter_context(tc.tile_pool(name="st", bufs=2))
        dnp = ctx.enter_context(tc.tile_pool(name="dn", bufs=2))
        rcbp = ctx.enter_context(tc.tile_pool(name="rcb", bufs=2))
        tmpp = ctx.enter_context(tc.tile_pool(name="tmp", bufs=2))
        osbp = ctx.enter_context(tc.tile_pool(name="osb", bufs=3))
        work = ctx.enter_context(
            tc.tile_pool(name="work", bufs=2, space="PSUM"))
        ctp = ctx.enter_context(
            tc.tile_pool(name="ctp", bufs=1, space="PSUM"))
        aux = ctx.enter_context(
            tc.tile_pool(name="aux", bufs=2, space="PSUM"))

        # --- resident SBUF tensors ---
        wq_sb = singles.tile([128, NKCH, LDIM], DT, tag="wq")
        wk_sb = singles.tile([128, NKCH, LDIM], DT, tag="wk")
        wv_sb = singles.tile([128, NKCH, LDIM], DT, tag="wv")
        wo_sb = singles.tile([LDIM, DIM], DT, tag="wo")
        qb_sb = singles.tile([LDIM, 1], F32, tag="qb")
        kb_sb = singles.tile([LDIM, 1], F32, tag="kb")
        vb_sb = singles.tile([LDIM, 1], F32, tag="vb")
        mask_sb = singles.tile([128, BS * NKT], F32, tag="mask")
        ident = singles.tile([128, 128], F32, tag="ident")
        qt_sb = singles.tile([128, NTOK], DT, tag="qt")
        kt_sb = singles.tile([128, NTOK], DT, tag="kt")
        ct_sb = singles.tile([128, NTOK], DT, tag="ct")
        # V stationary per (batch, kt): [h0 dims 64 | h1 dims 64] so the two
        # heads' PV matmuls col-tile at (0,0)/(0,64) and co-execute
        v_aug = singles.tile([128, BS, NKT, 128], DT, tag="vaug")
        # all-ones stationary column on partitions 0/64 for the K=1 broadcast
        # matmuls that replicate each head's softmax denominator row
        ones_col = singles.tile([HEAD + 1, HEAD], DT, tag="ones")
        # all-ones [128,1] stationary for the denominator row-sum matmuls
        ones128 = singles.tile([128, 1], DT, tag="ones128")

        # weights first and alone so group 0's matmuls aren't starved behind
        # the rest of the initial DMA soup; everything else is issued after
        # group 0's xt triggers (see below)
        nc.sync.dma_start(wq_sb[:], wq[:])
        nc.gpsimd.dma_start(wk_sb[:], wk[:])
        nc.scalar.dma_start(wv_sb[:], wv[:])
        nc.sync.dma_start(qb_sb[:], qb[:])
        nc.sync.dma_start(kb_sb[:], kb[:])
        nc.sync.dma_start(vb_sb[:], vb[:])
        nc.sync.dma_start(mask_sb[:], maskd[:])
        masks.make_identity(nc, ident[:])
        nc.vector.memset(ones_col[:], 1.0)
        nc.vector.memset(ones128[:], 1.0)

        dma_engines = [nc.sync, nc.gpsimd, nc.scalar]

        # --- phase 1: QKV projections + PE-transpose of V ---
        # xt is streamed as [128, 1024] two-group tiles so each SWDGE trigger
        # (~10ns/descriptor on the issuing queue) covers twice the data.
        xt_tiles = {}
        vt_tiles = {}
        for g in range(NTT):
            gs = slice(512 * g, 512 * (g + 1))
            b = g // (NTT // BS)
            s_qk = work.tile([128, 2, 512], F32, tag="w")
            v_ps = aux.tile([128, 512], F32, tag="aux")
            for c in range(NKCH):
                if g % 2 == 0:
                    xt_t = xpool.tile([128, 1024], DT, tag="xt")
                    dma_engines[(g * NKCH // 2 + c) % 3].dma_start(
                        xt_t[:], xt[:, c, 512 * g:512 * (g + 2)])
                    xt_tiles[c] = xt_t
                xt_v = xt_tiles[c][:, 512 * (g % 2):512 * (g % 2 + 1)]
                st_, sp_ = (c == 0), (c == NKCH - 1)
                nc.tensor.matmul(s_qk[:, 0, :], wq_sb[:, c, :], xt_v,
                                 start=st_, stop=sp_)
                nc.tensor.matmul(s_qk[:, 1, :], wk_sb[:, c, :], xt_v,
                                 start=st_, stop=sp_)
                nc.tensor.matmul(v_ps[:], wv_sb[:, c, :], xt_v,
                                 start=st_, stop=sp_)
            if g == 0:
                nc.scalar.dma_start(wo_sb[:], wo[:])
            vt_t = vtp.tile([128, 512], F32, tag="vt")
            nc.vector.tensor_scalar_add(vt_t[:], v_ps[:], vb_sb[:, 0:1])
            nc.vector.tensor_scalar_add(qt_sb[:, gs], s_qk[:, 0, :],
                                        qb_sb[:, 0:1])
            nc.vector.tensor_scalar_add(kt_sb[:, gs], s_qk[:, 1, :],
                                        kb_sb[:, 0:1])
            for k4 in range(4):
                kt_v = (g % (NTT // BS)) * 4 + k4
                tr = aux.tile([128, 128], F32, tag="aux")
                nc.tensor.matmul(tr[:], vt_t[:, 128 * k4:128 * (k4 + 1)],
                                 ident[:], is_transpose=True,
                                 start=True, stop=True)
                nc.vector.tensor_copy(v_aug[:, b, kt_v, :], tr[:])

        # --- phase 2: attention, 3-stage software pipeline over qtiles ---
        # iter w: scores+exp(w) | PV(w-1) | normalize+out-proj(w-2)
        st_tiles = {}
        ct_tiles = {}
        for w in range(NQT + 2):
            scoring = w < NQT
            pving = 1 <= w <= NQT
            if scoring:
                b = w // (NQT // BS)
                qs = slice(QTW * w, QTW * (w + 1))
                st_cur = stp.tile([128, NKT, 2, QTW], DT, tag="st")
                st_tiles[w] = st_cur
            if pving:
                q1 = w - 1
                b1 = q1 // (NQT // BS)
                st_prev = st_tiles[q1]
                ct_pair = ctp.tile([128, QTW], F32, tag="cta")
                dn_acc = ctp.tile([HEAD + 1, QTW], F32, tag="dnacc")
                ct_tiles[q1] = (ct_pair, dn_acc)

            for kt_i in range(NKT):
                if scoring:
                    s_ps = work.tile([128, 2, 512], F32, tag="w")
                    ks = slice(QLEN * b + 128 * kt_i,
                               QLEN * b + 128 * (kt_i + 1))
                    for h in range(2):
                        hs = slice(HEAD * h, HEAD * (h + 1))
                        nc.tensor.matmul(s_ps[:, h, :], kt_sb[hs, ks],
                                         qt_sb[hs, qs], start=True, stop=True,
                                         tile_position=(HEAD * h, 0))
                    m_ap = mask_sb[:, b * NKT + kt_i:b * NKT + kt_i + 1]
                    nc.scalar.activation(
                        st_cur[:, kt_i, :, :].rearrange("p a b -> p (a b)"),
                        s_ps[:].rearrange("p a b -> p (a b)"),
                        mybir.ActivationFunctionType.Exp, bias=m_ap)
                if pving:
                    st0, sp0 = (kt_i == 0), (kt_i == NKT - 1)
                    # PV pair col-tiled at (0,0)/(0,64) -> co-executes; the
                    # denominator row-sum pair col-tiles the same way.
                    for h in range(2):
                        nc.tensor.matmul(
                            ct_pair[HEAD * h:HEAD * (h + 1), :],
                            v_aug[:, b1, kt_i, HEAD * h:HEAD * (h + 1)],
                            st_prev[:, kt_i, h, :],
                            start=st0, stop=sp0,
                            tile_position=(0, HEAD * h),
                            skip_group_check=True)
                    for h in range(2):
                        nc.tensor.matmul(
                            dn_acc[HEAD * h:HEAD * h + 1, :],
                            ones128[:], st_prev[:, kt_i, h, :],
                            start=st0, stop=sp0,
                            tile_position=(0, HEAD * h),
                            skip_group_check=True)
                # normalize qtile w-2: emitted after kt 0's matmuls so the PE
                # queue has work ahead of the broadcast matmuls while the DVE
                # denominator copies run.
                if 2 <= w and kt_i == 0:
                    q2 = w - 2
                    qs2 = slice(QTW * q2, QTW * (q2 + 1))
                    ct_pair2, dn_acc2 = ct_tiles.pop(q2)
                    dn_sb = dnp.tile([HEAD + 1, QTW], DT, tag="dn")
                    nc.vector.tensor_copy(dn_sb[0:1, :], dn_acc2[0:1, :])
                    nc.vector.tensor_copy(dn_sb[HEAD:HEAD + 1, :],
                                          dn_acc2[HEAD:HEAD + 1, :])
                    rcb_ps = aux.tile([128, QTW], F32, tag="aux")
                    nc.tensor.matmul(rcb_ps[0:HEAD, :], ones_col[0:1, :],
                                     dn_sb[0:1, :], start=True, stop=True,
                                     tile_position=(0, 0))
                    nc.tensor.matmul(rcb_ps[HEAD:128, :],
                                     ones_col[HEAD:HEAD + 1, :],
                                     dn_sb[HEAD:HEAD + 1, :],
                                     start=True, stop=True,
                                     tile_position=(HEAD, HEAD),
                                     skip_group_check=True)
                    # single-pass approximate reciprocal (~18 bits, plenty for
                    # bf16 context) -- the exact multi-pass reciprocal costs
                    # ~3.3us at [128,512] on HW
                    rcb_t = rcbp.tile([128, QTW], F32, tag="rcb")
                    nc.vector.reciprocal_approx_fast(rcb_t[:], rcb_ps[:])
                    ev_t = tmpp.tile([128, QTW], DT, tag="ev")
                    nc.vector.tensor_copy(ev_t[:], ct_pair2[:])
                    nc.vector.tensor_mul(ct_sb[:, qs2], ev_t[:], rcb_t[:])
                # out-projection for qtile w-2, one token tile per 4 kt;
                # PSUM comes from the aux pool so the scores rotation is
                # never blocked behind an out-proj eviction.
                if 2 <= w and kt_i % 4 == 3:
                    t = kt_i // 4
                    tok0 = QTW * (w - 2) + 128 * t
                    lhs = ct_sb[:, tok0:tok0 + 128]
                    o_sb = osbp.tile([128, 2, 512], DT, tag="osb")
                    for half in range(2):
                        o_ps = aux.tile([128, 512], F32, tag="aux")
                        nc.tensor.matmul(o_ps[:], lhs,
                                         wo_sb[:, 512 * half:512 * (half + 1)],
                                         start=True, stop=True)
                        nc.vector.tensor_copy(o_sb[:, half, :], o_ps[:])
                    nc.sync.dma_start(
                        out[tok0:tok0 + 128, :],
                        o_sb[:].rearrange("p a b -> p (a b)"))
            if scoring:
                pass

    nc.compile()
    return nc


def shard_inputs(input, mask, q_w, q_b, k_w, k_b, v_w, v_b, o_w, o_b):
    x = np.asarray(input, np.float32)
    # [128, c, tok] layout: partition p of chunk c is feature row 128c+p
    xt = np.ascontiguousarray(
        x.T.reshape(NKCH, 128, NTOK).transpose(1, 0, 2)).astype(NPDT)
    m = np.asarray(mask, np.float32).reshape(BS, NKT, 128)
    maskd = np.ascontiguousarray(m.transpose(2, 0, 1).reshape(128, BS * NKT))
    scale = 1.0 / math.sqrt(HEAD)

    def warr(w):  # [DIM, LDIM] -> [128, c, LDIM]
        return np.ascontiguousarray(
            w.reshape(NKCH, 128, LDIM).transpose(1, 0, 2)).astype(NPDT)

    in_maps = []
    for c in range(NCORES):
        L = slice(LDIM * c, LDIM * (c + 1))
        in_maps.append({
            "xt": xt,
            "wq": warr(np.ascontiguousarray((q_w[L, :] * scale).T)),
            "wk": warr(np.ascontiguousarray(k_w[L, :].T)),
            "wv": warr(np.ascontiguousarray(v_w[L, :].T)),
            "wo": np.ascontiguousarray(o_w[:, L].T).astype(NPDT),
            "qb": (q_b[L] * scale).astype(np.float32).reshape(LDIM, 1),
            "kb": k_b[L].astype(np.float32).reshape(LDIM, 1),
            "vb": v_b[L].astype(np.float32).reshape(LDIM, 1),
            "maskd": maskd,
        })
    return in_maps


def run(in_maps, **kw):
    if "nc" not in _cache:
        _cache["nc"] = build_program()
    return run_bass_kernel_spmd(_cache["nc"], in_maps,
                                core_ids=list(range(NCORES)), **kw)


def gather_output(results, o_b):
    acc = np.zeros((NTOK, DIM), np.float32)
    for r in results:
        acc += np.asarray(r["out"], np.float32)
    acc += np.asarray(o_b, np.float32)[None, :]
    return acc


def kernel(input, mask, q_w, q_b, k_w, k_b, v_w, v_b, o_w, o_b,
           bs=BS, qlen=QLEN):
    assert int(bs) == BS and int(qlen) == QLEN
    in_maps = shard_inputs(np.asarray(input), np.asarray(mask),
                           np.asarray(q_w), np.asarray(q_b),
                           np.asarray(k_w), np.asarray(k_b),
                           np.asarray(v_w), np.asarray(v_b),
                           np.asarray(o_w), np.asarray(o_b))
    res = run(in_maps)
    return gather_output(res.results, o_b)


# revision 56
# speedup vs baseline: 1.0356x; 1.0356x over previous
"""MultiHeadAttention TRN2 kernel: tensor-parallel over heads across 8 NeuronCores.

Problem (hardcoded): BS=2, QLEN=2048, DIM=1024, NHEADS=16, HEAD=64.
  q = split_heads(x @ q_w.T + q_b) / sqrt(64)
  s = q @ k.T + mask ; w = softmax(s) ; ctx = w @ v
  out = merge_heads(ctx) @ o_w.T + o_b

Sharding: core c computes heads {2c, 2c+1} (rows 128c:128c+128 of q/k/v weights,
cols 128c:128c+128 of o_w).  Each core emits a full-shape bf16 partial of the
output projection; the host sums the 8 partials in f32 and adds o_b.

v2 design notes (vs the v1 baseline at 475us):
- All DRAM operands are pre-arranged on the host so every dma_start moves big
  contiguous rows (the SWDGE trigger costs ~10ns/descriptor on the issuing
  engine queue); DMA triggers are spread across sync/gpsimd/vector/scalar.
- V is transposed on the PE (transpose-mode matmul, f32 PSUM out) instead of
  descriptor-exploding DMA transposes.
- V stationary tiles carry an extra all-ones column, so each PV matmul also
  produces the softmax denominator in PSUM partition 64 for free -> the 25% of
  matmuls the baseline spent on ones-matmul row sums are gone.
- Attention is ACT-paced: exp of scores is the hard floor (~16.8M elems/core
  through the scalar engine).  The qtile loop is software-pipelined 3 deep
  (scores+exp for qtile i, PV for i-1, normalize+out-proj for i-2) with a
  double-buffered st so the PE/DVE work hides under the exp stream.
- exp is fused over both heads of one k-tile ([128, 2x512] PSUM AP, mask fed
  through the activation bias port).
"""

import sys

if "/opt/trn_rl_repo" not in sys.path:
    sys.path.insert(0, "/opt/trn_rl_repo")

import math
from contextlib import ExitStack

import ml_dtypes
import numpy as np

import concourse.bass as bass
import concourse.tile as tile
from concourse import bacc, masks, mybir
from concourse.bass_utils import run_bass_kernel_spmd


# ---- problem constants ----
BS, QLEN, DIM, NHEADS = 2, 2048, 1024, 16
HEAD = DIM // NHEADS            # 64
NTOK = BS * QLEN                # 4096
NCORES = 8
HPC = NHEADS // NCORES          # 2 heads per core
LDIM = HPC * HEAD               # 128 local dims per core
NKCH = DIM // 128               # 8 contraction chunks for projections
NTT = NTOK // 512               # 8 token groups of 512 for projections
NKT = QLEN // 128               # 16 key tiles per batch
QTW = 512                       # query tile width for attention
NQT = NTOK // QTW               # 8 query tiles total (4 per batch)

DT = mybir.dt.bfloat16          # matmul compute dtype
NPDT = ml_dtypes.bfloat16
F32 = mybir.dt.float32

_cache = {}


def build_program():
    """Build + compile the single-core SPMD Bass program."""
    nc = bacc.Bacc("TRN2", target_bir_lowering=False, debug=False,
                   num_devices=NCORES)

    xt = nc.dram_tensor("xt", [128, NKCH, NTOK], DT, kind="ExternalInput").ap()
    wq = nc.dram_tensor("wq", [128, NKCH, LDIM], DT, kind="ExternalInput").ap()
    wk = nc.dram_tensor("wk", [128, NKCH, LDIM], DT, kind="ExternalInput").ap()
    wv = nc.dram_tensor("wv", [128, NKCH, LDIM], DT, kind="ExternalInput").ap()
    wo = nc.dram_tensor("wo", [LDIM, DIM], DT, kind="ExternalInput").ap()
    qb = nc.dram_tensor("qb", [LDIM, 1], F32, kind="ExternalInput").ap()
    kb = nc.dram_tensor("kb", [LDIM, 1], F32, kind="ExternalInput").ap()
    vb = nc.dram_tensor("vb", [LDIM, 1], F32, kind="ExternalInput").ap()
    maskd = nc.dram_tensor("maskd", [128, BS * NKT], F32,
                           kind="ExternalInput").ap()
    out = nc.dram_tensor("out", [NTOK, DIM], DT, kind="ExternalOutput").ap()

    with tile.TileContext(nc) as tc, ExitStack() as ctx:
        singles = ctx.enter_context(tc.tile_pool(name="singles", bufs=1))
        xpool = ctx.enter_context(tc.tile_pool(name="xs", bufs=12))
        vtp = ctx.enter_context(tc.tile_pool(name="vt", bufs=8))
        stp = ctx.enter_context(tc.tile_pool(name="st", bufs=2))
        dnp = ctx.enter_context(tc.tile_pool(name="dn", bufs=2))
        rcbp = ctx.enter_context(tc.tile_pool(name="rcb", bufs=2))
        tmpp = ctx.enter_context(tc.tile_pool(name="tmp", bufs=2))
        osbp = ctx.enter_context(tc.tile_pool(name="osb", bufs=3))
        work = ctx.enter_context(
            tc.tile_pool(name="work", bufs=2, space="PSUM"))
        ctp = ctx.enter_context(
            tc.tile_pool(name="ctp", bufs=1, space="PSUM"))
        aux = ctx.enter_context(
            tc.tile_pool(name="aux", bufs=2, space="PSUM"))

        # --- resident SBUF tensors ---
        wq_sb = singles.tile([128, NKCH, LDIM], DT, tag="wq")
        wk_sb = singles.tile([128, NKCH, LDIM], DT, tag="wk")
        wv_sb = singles.tile([128, NKCH, LDIM], DT, tag="wv")
        wo_sb = singles.tile([LDIM, DIM], DT, tag="wo")
        qb_sb = singles.tile([LDIM, 1], F32, tag="qb")
        kb_sb = singles.tile([LDIM, 1], F32, tag="kb")
        vb_sb = singles.tile([LDIM, 1], F32, tag="vb")
        mask_sb = singles.tile([128, BS * NKT], F32, tag="mask")
        ident = singles.tile([128, 128], F32, tag="ident")
        qt_sb = singles.tile([128, NTOK], DT, tag="qt")
        kt_sb = singles.tile([128, NTOK], DT, tag="kt")
        ct_sb = singles.tile([128, NTOK], DT, tag="ct")
        # V stationary per (batch, kt): [h0 dims 64 | h1 dims 64] so the two
        # heads' PV matmuls col-tile at (0,0)/(0,64) and co-execute
        v_aug = singles.tile([128, BS, NKT, 128], DT, tag="vaug")
        # all-ones stationary column on partitions 0/64 for the K=1 broadcast
        # matmuls that replicate each head's softmax denominator row
        ones_col = singles.tile([HEAD + 1, HEAD], DT, tag="ones")
        # all-ones [128,1] stationary for the denominator row-sum matmuls
        ones128 = singles.tile([128, 1], DT, tag="ones128")

        # weights first and alone so group 0's matmuls aren't starved behind
        # the rest of the initial DMA soup; everything else is issued after
        # group 0's xt triggers (see below)
        nc.sync.dma_start(wq_sb[:], wq[:])
        nc.gpsimd.dma_start(wk_sb[:], wk[:])
        nc.scalar.dma_start(wv_sb[:], wv[:])
        nc.sync.dma_start(qb_sb[:], qb[:])
        nc.sync.dma_start(kb_sb[:], kb[:])
        nc.sync.dma_start(vb_sb[:], vb[:])
        nc.sync.dma_start(mask_sb[:], maskd[:])
        masks.make_identity(nc, ident[:])
        nc.vector.memset(ones_col[:], 1.0)
        nc.vector.memset(ones128[:], 1.0)

        dma_engines = [nc.sync, nc.gpsimd, nc.scalar]

        # --- phase 1: QKV projections + PE-transpose of V ---
        # xt is streamed as [128, 1024] two-group tiles so each SWDGE trigger
        # (~10ns/descriptor on the issuing queue) covers twice the data.
        xt_tiles = {}
        vt_tiles = {}
        for g in range(NTT):
            gs = slice(512 * g, 512 * (g + 1))
            b = g // (NTT // BS)
            s_qk = work.tile([128, 2, 512], F32, tag="w")
            v_ps = aux.tile([128, 512], F32, tag="aux")
            for c in range(NKCH):
                if g % 2 == 0:
                    xt_t = xpool.tile([128, 1024], DT, tag="xt")
                    dma_engines[(g * NKCH // 2 + c) % 3].dma_start(
                        xt_t[:], xt[:, c, 512 * g:512 * (g + 2)])
                    xt_tiles[c] = xt_t
                xt_v = xt_tiles[c][:, 512 * (g % 2):512 * (g % 2 + 1)]
                st_, sp_ = (c == 0), (c == NKCH - 1)
                nc.tensor.matmul(s_qk[:, 0, :], wq_sb[:, c, :], xt_v,
                                 start=st_, stop=sp_)
                nc.tensor.matmul(s_qk[:, 1, :], wk_sb[:, c, :], xt_v,
                                 start=st_, stop=sp_)
                nc.tensor.matmul(v_ps[:], wv_sb[:, c, :], xt_v,
                                 start=st_, stop=sp_)
            if g == 0:
                nc.scalar.dma_start(wo_sb[:], wo[:])
            vt_t = vtp.tile([128, 512], F32, tag="vt")
            nc.vector.tensor_scalar_add(vt_t[:], v_ps[:], vb_sb[:, 0:1])
            nc.vector.tensor_scalar_add(qt_sb[:, gs], s_qk[:, 0, :],
                                        qb_sb[:, 0:1])
            nc.vector.tensor_scalar_add(kt_sb[:, gs], s_qk[:, 1, :],
                                        kb_sb[:, 0:1])
            for k4 in range(4):
                kt_v = (g % (NTT // BS)) * 4 + k4
                tr = aux.tile([128, 128], F32, tag="aux")
                nc.tensor.matmul(tr[:], vt_t[:, 128 * k4:128 * (k4 + 1)],
                                 ident[:], is_transpose=True,
                                 start=True, stop=True)
                nc.vector.tensor_copy(v_aug[:, b, kt_v, :], tr[:])

        # --- phase 2: attention, 3-stage software pipeline over qtiles ---
        # iter w: scores+exp(w) | PV(w-1) | normalize+out-proj(w-2)
        st_tiles = {}
        ct_tiles = {}
        for w in range(NQT + 2):
            scoring = w < NQT
            pving = 1 <= w <= NQT
            if scoring:
                b = w // (NQT // BS)
                qs = slice(QTW * w, QTW * (w + 1))
                st_cur = stp.tile([128, NKT, 2, QTW], DT, tag="st")
                st_tiles[w] = st_cur
            if pving:
                q1 = w - 1
                b1 = q1 // (NQT // BS)
                st_prev = st_tiles[q1]
                ct_pair = ctp.tile([128, QTW], F32, tag="cta")
                dn_acc = ctp.tile([HEAD + 1, QTW], F32, tag="dnacc")
                ct_tiles[q1] = (ct_pair, dn_acc)

            for kt_i in range(NKT):
                if scoring:
                    s_ps = work.tile([128, 2, 512], F32, tag="w")
                    ks = slice(QLEN * b + 128 * kt_i,
                               QLEN * b + 128 * (kt_i + 1))
                    for h in range(2):
                        hs = slice(HEAD * h, HEAD * (h + 1))
                        nc.tensor.matmul(s_ps[:, h, :], kt_sb[hs, ks],
                                         qt_sb[hs, qs], start=True, stop=True,
                                         tile_position=(HEAD * h, 0))
                    m_ap = mask_sb[:, b * NKT + kt_i:b * NKT + kt_i + 1]
                    nc.scalar.activation(st_cur[:, kt_i, :, :], s_ps[:],
                                         mybir.ActivationFunctionType.Exp,
                                         bias=m_ap)
                if pving:
                    st0, sp0 = (kt_i == 0), (kt_i == NKT - 1)
                    # PV pair col-tiled at (0,0)/(0,64) -> co-executes; the
                    # denominator row-sum pair col-tiles the same way.
                    for h in range(2):
                        nc.tensor.matmul(
                            ct_pair[HEAD * h:HEAD * (h + 1), :],
                            v_aug[:, b1, kt_i, HEAD * h:HEAD * (h + 1)],
                            st_prev[:, kt_i, h, :],
                            start=st0, stop=sp0,
                            tile_position=(0, HEAD * h),
                            skip_group_check=True)
                    for h in range(2):
                        nc.tensor.matmul(
                            dn_acc[HEAD * h:HEAD * h + 1, :],
                            ones128[:], st_prev[:, kt_i, h, :],
                            start=st0, stop=sp0,
                            tile_position=(0, HEAD * h),
                            skip_group_check=True)
                # normalize qtile w-2: emitted after kt 0's matmuls so the PE
                # queue has work ahead of the broadcast matmuls while the DVE
                # denominator copies run.
                if 2 <= w and kt_i == 0:
                    q2 = w - 2
                    qs2 = slice(QTW * q2, QTW * (q2 + 1))
                    ct_pair2, dn_acc2 = ct_tiles.pop(q2)
                    dn_sb = dnp.tile([HEAD + 1, QTW], DT, tag="dn")
                    nc.vector.tensor_copy(dn_sb[0:1, :], dn_acc2[0:1, :])
                    nc.vector.tensor_copy(dn_sb[HEAD:HEAD + 1, :],
                                          dn_acc2[HEAD:HEAD + 1, :])
                    rcb_ps = aux.tile([128, QTW], F32, tag="aux")
                    nc.tensor.matmul(rcb_ps[0:HEAD, :], ones_col[0:1, :],
                                     dn_sb[0:1, :], start=True, stop=True,
                                     tile_position=(0, 0))
                    nc.tensor.matmul(rcb_ps[HEAD:128, :],
                                     ones_col[HEAD:HEAD + 1, :],
                                     dn_sb[HEAD:HEAD + 1, :],
                                     start=True, stop=True,
                                     tile_position=(HEAD, HEAD),
                                     skip_group_check=True)
                    # single-pass approximate reciprocal (~18 bits, plenty for
                    # bf16 context) -- the exact multi-pass reciprocal costs
                    # ~3.3us at [128,512] on HW
                    rcb_t = rcbp.tile([128, QTW], F32, tag="rcb")
                    nc.vector.reciprocal_approx_fast(rcb_t[:], rcb_ps[:])
                    ev_t = tmpp.tile([128, QTW], DT, tag="ev")
                    nc.vector.tensor_copy(ev_t[:], ct_pair2[:])
                    nc.vector.tensor_mul(ct_sb[:, qs2], ev_t[:], rcb_t[:])
                # out-projection for qtile w-2, one token tile per 4 kt;
                # PSUM comes from the aux pool so the scores rotation is
                # never blocked behind an out-proj eviction.
                if 2 <= w and kt_i % 4 == 3:
                    t = kt_i // 4
                    tok0 = QTW * (w - 2) + 128 * t
                    lhs = ct_sb[:, tok0:tok0 + 128]
                    o_sb = osbp.tile([128, 2, 512], DT, tag="osb")
                    for half in range(2):
                        o_ps = aux.tile([128, 512], F32, tag="aux")
                        nc.tensor.matmul(o_ps[:], lhs,
                                         wo_sb[:, 512 * half:512 * (half + 1)],
                                         start=True, stop=True)
                        nc.vector.tensor_copy(o_sb[:, half, :], o_ps[:])
                    nc.sync.dma_start(
                        out[tok0:tok0 + 128, :],
                        o_sb[:].rearrange("p a b -> p (a b)"))
            if scoring:
                pass

    nc.compile()
    return nc


def shard_inputs(input, mask, q_w, q_b, k_w, k_b, v_w, v_b, o_w, o_b):
    x = np.asarray(input, np.float32)
    # [128, c, tok] layout: partition p of chunk c is feature row 128c+p
    xt = np.ascontiguousarray(
        x.T.reshape(NKCH, 128, NTOK).transpose(1, 0, 2)).astype(NPDT)
    m = np.asarray(mask, np.float32).reshape(BS, NKT, 128)
    maskd = np.ascontiguousarray(m.transpose(2, 0, 1).reshape(128, BS * NKT))
    scale = 1.0 / math.sqrt(HEAD)

    def warr(w):  # [DIM, LDIM] -> [128, c, LDIM]
        return np.ascontiguousarray(
            w.reshape(NKCH, 128, LDIM).transpose(1, 0, 2)).astype(NPDT)

    in_maps = []
    for c in range(NCORES):
        L = slice(LDIM * c, LDIM * (c + 1))
        in_maps.append({
            "xt": xt,
            "wq": warr(np.ascontiguousarray((q_w[L, :] * scale).T)),
            "wk": warr(np.ascontiguousarray(k_w[L, :].T)),
            "wv": warr(np.ascontiguousarray(v_w[L, :].T)),
            "wo": np.ascontiguousarray(o_w[:, L].T).astype(NPDT),
            "qb": (q_b[L] * scale).astype(np.float32).reshape(LDIM, 1),
            "kb": k_b[L].astype(np.float32).reshape(LDIM, 1),
            "vb": v_b[L].astype(np.float32).reshape(LDIM, 1),
            "maskd": maskd,
        })
    return in_maps


def run(in_maps, **kw):
    if "nc" not in _cache:
        _cache["nc"] = build_program()
    return run_bass_kernel_spmd(_cache["nc"], in_maps,
                                core_ids=list(range(NCORES)), **kw)


def gather_output(results, o_b):
    acc = np.zeros((NTOK, DIM), np.float32)
    for r in results:
        acc += np.asarray(r["out"], np.float32)
    acc += np.asarray(o_b, np.float32)[None, :]
    return acc


def kernel(input, mask, q_w, q_b, k_w, k_b, v_w, v_b, o_w, o_b,
           bs=BS, qlen=QLEN):
    assert int(bs) == BS and int(qlen) == QLEN
    in_maps = shard_inputs(np.asarray(input), np.asarray(mask),
                           np.asarray(q_w), np.asarray(q_b),
                           np.asarray(k_w), np.asarray(k_b),
                           np.asarray(v_w), np.asarray(v_b),
                           np.asarray(o_w), np.asarray(o_b))
    res = run(in_maps)
    return gather_output(res.results, o_b)


# revision 58
# speedup vs baseline: 1.0549x; 1.0186x over previous
"""MultiHeadAttention TRN2 kernel: tensor-parallel over heads across 8 NeuronCores.

Problem (hardcoded): BS=2, QLEN=2048, DIM=1024, NHEADS=16, HEAD=64.
  q = split_heads(x @ q_w.T + q_b) / sqrt(64)
  s = q @ k.T + mask ; w = softmax(s) ; ctx = w @ v
  out = merge_heads(ctx) @ o_w.T + o_b

Sharding: core c computes heads {2c, 2c+1} (rows 128c:128c+128 of q/k/v weights,
cols 128c:128c+128 of o_w).  Each core emits a full-shape bf16 partial of the
output projection; the host sums the 8 partials in f32 and adds o_b.

v2 design notes (vs the v1 baseline at 475us):
- All DRAM operands are pre-arranged on the host so every dma_start moves big
  contiguous rows (the SWDGE trigger costs ~10ns/descriptor on the issuing
  engine queue); DMA triggers are spread across sync/gpsimd/vector/scalar.
- V is transposed on the PE (transpose-mode matmul, f32 PSUM out) instead of
  descriptor-exploding DMA transposes.
- V stationary tiles carry an extra all-ones column, so each PV matmul also
  produces the softmax denominator in PSUM partition 64 for free -> the 25% of
  matmuls the baseline spent on ones-matmul row sums are gone.
- Attention is ACT-paced: exp of scores is the hard floor (~16.8M elems/core
  through the scalar engine).  The qtile loop is software-pipelined 3 deep
  (scores+exp for qtile i, PV for i-1, normalize+out-proj for i-2) with a
  double-buffered st so the PE/DVE work hides under the exp stream.
- exp is fused over both heads of one k-tile ([128, 2x512] PSUM AP, mask fed
  through the activation bias port).
"""

import sys

if "/opt/trn_rl_repo" not in sys.path:
    sys.path.insert(0, "/opt/trn_rl_repo")

import math
from contextlib import ExitStack

import ml_dtypes
import numpy as np

import concourse.bass as bass
import concourse.tile as tile
from concourse import bacc, masks, mybir
from concourse.bass_utils import run_bass_kernel_spmd


# ---- problem constants ----
BS, QLEN, DIM, NHEADS = 2, 2048, 1024, 16
HEAD = DIM // NHEADS            # 64
NTOK = BS * QLEN                # 4096
NCORES = 8
HPC = NHEADS // NCORES          # 2 heads per core
LDIM = HPC * HEAD               # 128 local dims per core
NKCH = DIM // 128               # 8 contraction chunks for projections
NTT = NTOK // 512               # 8 token groups of 512 for projections
NKT = QLEN // 128               # 16 key tiles per batch
QTW = 512                       # query tile width for attention
NQT = NTOK // QTW               # 8 query tiles total (4 per batch)

DT = mybir.dt.bfloat16          # matmul compute dtype
NPDT = ml_dtypes.bfloat16
F32 = mybir.dt.float32

_cache = {}


def build_program():
    """Build + compile the single-core SPMD Bass program."""
    nc = bacc.Bacc("TRN2", target_bir_lowering=False, debug=False,
                   num_devices=NCORES)

    xt = nc.dram_tensor("xt", [128, NKCH, NTOK], DT, kind="ExternalInput").ap()
    wq = nc.dram_tensor("wq", [128, NKCH, LDIM], DT, kind="ExternalInput").ap()
    wk = nc.dram_tensor("wk", [128, NKCH, LDIM], DT, kind="ExternalInput").ap()
    wv = nc.dram_tensor("wv", [128, NKCH, LDIM], DT, kind="ExternalInput").ap()
    wo = nc.dram_tensor("wo", [LDIM, DIM], DT, kind="ExternalInput").ap()
    qb = nc.dram_tensor("qb", [LDIM, 1], F32, kind="ExternalInput").ap()
    kb = nc.dram_tensor("kb", [LDIM, 1], F32, kind="ExternalInput").ap()
    vb = nc.dram_tensor("vb", [LDIM, 1], F32, kind="ExternalInput").ap()
    maskd = nc.dram_tensor("maskd", [128, BS * NKT], F32,
                           kind="ExternalInput").ap()
    out = nc.dram_tensor("out", [NTOK, DIM], DT, kind="ExternalOutput").ap()

    with tile.TileContext(nc) as tc, ExitStack() as ctx:
        singles = ctx.enter_context(tc.tile_pool(name="singles", bufs=1))
        xpool = ctx.enter_context(tc.tile_pool(name="xs", bufs=16))
        vtp = ctx.enter_context(tc.tile_pool(name="vt", bufs=8))
        stp = ctx.enter_context(tc.tile_pool(name="st", bufs=2))
        dnp = ctx.enter_context(tc.tile_pool(name="dn", bufs=2))
        rcbp = ctx.enter_context(tc.tile_pool(name="rcb", bufs=2))
        tmpp = ctx.enter_context(tc.tile_pool(name="tmp", bufs=2))
        osbp = ctx.enter_context(tc.tile_pool(name="osb", bufs=4))
        work = ctx.enter_context(
            tc.tile_pool(name="work", bufs=2, space="PSUM"))
        ctp = ctx.enter_context(
            tc.tile_pool(name="ctp", bufs=1, space="PSUM"))
        aux = ctx.enter_context(
            tc.tile_pool(name="aux", bufs=2, space="PSUM"))

        # --- resident SBUF tensors ---
        wq_sb = singles.tile([128, NKCH, LDIM], DT, tag="wq")
        wk_sb = singles.tile([128, NKCH, LDIM], DT, tag="wk")
        wv_sb = singles.tile([128, NKCH, LDIM], DT, tag="wv")
        wo_sb = singles.tile([LDIM, DIM], DT, tag="wo")
        qb_sb = singles.tile([LDIM, 1], F32, tag="qb")
        kb_sb = singles.tile([LDIM, 1], F32, tag="kb")
        vb_sb = singles.tile([LDIM, 1], F32, tag="vb")
        mask_sb = singles.tile([128, BS * NKT], F32, tag="mask")
        ident = singles.tile([128, 128], F32, tag="ident")
        qt_sb = singles.tile([128, NTOK], DT, tag="qt")
        kt_sb = singles.tile([128, NTOK], DT, tag="kt")
        ct_sb = singles.tile([128, NTOK], DT, tag="ct")
        # V stationary per (batch, kt): [h0 dims 64 | h1 dims 64] so the two
        # heads' PV matmuls col-tile at (0,0)/(0,64) and co-execute
        v_aug = singles.tile([128, BS, NKT, 128], DT, tag="vaug")
        # all-ones stationary column on partitions 0/64 for the K=1 broadcast
        # matmuls that replicate each head's softmax denominator row
        ones_col = singles.tile([HEAD + 1, HEAD], DT, tag="ones")
        # all-ones [128,1] stationary for the denominator row-sum matmuls
        ones128 = singles.tile([128, 1], DT, tag="ones128")

        # weights first and alone so group 0's matmuls aren't starved behind
        # the rest of the initial DMA soup; everything else is issued after
        # group 0's xt triggers (see below)
        nc.sync.dma_start(wq_sb[:], wq[:])
        nc.gpsimd.dma_start(wk_sb[:], wk[:])
        nc.scalar.dma_start(wv_sb[:], wv[:])
        nc.sync.dma_start(qb_sb[:], qb[:])
        nc.sync.dma_start(kb_sb[:], kb[:])
        nc.sync.dma_start(vb_sb[:], vb[:])
        nc.sync.dma_start(mask_sb[:], maskd[:])
        masks.make_identity(nc, ident[:])
        nc.vector.memset(ones_col[:], 1.0)
        nc.vector.memset(ones128[:], 1.0)

        dma_engines = [nc.sync, nc.gpsimd, nc.scalar]

        # --- phase 1: QKV projections + PE-transpose of V ---
        # xt is streamed as [128, 1024] two-group tiles so each SWDGE trigger
        # (~10ns/descriptor on the issuing queue) covers twice the data.
        xt_tiles = {}
        vt_tiles = {}
        for g in range(NTT):
            gs = slice(512 * g, 512 * (g + 1))
            b = g // (NTT // BS)
            s_qk = work.tile([128, 2, 512], F32, tag="w")
            v_ps = aux.tile([128, 512], F32, tag="aux")
            for c in range(NKCH):
                if g % 2 == 0:
                    xt_t = xpool.tile([128, 1024], DT, tag="xt")
                    dma_engines[(g * NKCH // 2 + c) % 3].dma_start(
                        xt_t[:], xt[:, c, 512 * g:512 * (g + 2)])
                    xt_tiles[c] = xt_t
                xt_v = xt_tiles[c][:, 512 * (g % 2):512 * (g % 2 + 1)]
                st_, sp_ = (c == 0), (c == NKCH - 1)
                nc.tensor.matmul(s_qk[:, 0, :], wq_sb[:, c, :], xt_v,
                                 start=st_, stop=sp_)
                nc.tensor.matmul(s_qk[:, 1, :], wk_sb[:, c, :], xt_v,
                                 start=st_, stop=sp_)
                nc.tensor.matmul(v_ps[:], wv_sb[:, c, :], xt_v,
                                 start=st_, stop=sp_)
            if g == 0:
                nc.scalar.dma_start(wo_sb[:], wo[:])
            vt_t = vtp.tile([128, 512], F32, tag="vt")
            nc.vector.tensor_scalar_add(vt_t[:], v_ps[:], vb_sb[:, 0:1])
            nc.vector.tensor_scalar_add(qt_sb[:, gs], s_qk[:, 0, :],
                                        qb_sb[:, 0:1])
            nc.vector.tensor_scalar_add(kt_sb[:, gs], s_qk[:, 1, :],
                                        kb_sb[:, 0:1])
            for k4 in range(4):
                kt_v = (g % (NTT // BS)) * 4 + k4
                tr = aux.tile([128, 128], F32, tag="aux")
                nc.tensor.matmul(tr[:], vt_t[:, 128 * k4:128 * (k4 + 1)],
                                 ident[:], is_transpose=True,
                                 start=True, stop=True)
                nc.vector.tensor_copy(v_aug[:, b, kt_v, :], tr[:])

        # --- phase 2: attention, 3-stage software pipeline over qtiles ---
        # iter w: scores+exp(w) | PV(w-1) | normalize+out-proj(w-2)
        st_tiles = {}
        ct_tiles = {}
        for w in range(NQT + 2):
            scoring = w < NQT
            pving = 1 <= w <= NQT
            if scoring:
                b = w // (NQT // BS)
                qs = slice(QTW * w, QTW * (w + 1))
                st_cur = stp.tile([128, NKT, 2, QTW], DT, tag="st")
                st_tiles[w] = st_cur
            if pving:
                q1 = w - 1
                b1 = q1 // (NQT // BS)
                st_prev = st_tiles[q1]
                ct_pair = ctp.tile([128, QTW], F32, tag="cta")
                dn_acc = ctp.tile([HEAD + 1, QTW], F32, tag="dnacc")
                ct_tiles[q1] = (ct_pair, dn_acc)

            for kt_i in range(NKT):
                if scoring:
                    s_ps = work.tile([128, 2, 512], F32, tag="w")
                    ks = slice(QLEN * b + 128 * kt_i,
                               QLEN * b + 128 * (kt_i + 1))
                    for h in range(2):
                        hs = slice(HEAD * h, HEAD * (h + 1))
                        nc.tensor.matmul(s_ps[:, h, :], kt_sb[hs, ks],
                                         qt_sb[hs, qs], start=True, stop=True,
                                         tile_position=(HEAD * h, 0))
                    m_ap = mask_sb[:, b * NKT + kt_i:b * NKT + kt_i + 1]
                    nc.scalar.activation(st_cur[:, kt_i, :, :], s_ps[:],
                                         mybir.ActivationFunctionType.Exp,
                                         bias=m_ap)
                if pving:
                    st0, sp0 = (kt_i == 0), (kt_i == NKT - 1)
                    # PV pair col-tiled at (0,0)/(0,64) -> co-executes; the
                    # denominator row-sum pair col-tiles the same way.
                    for h in range(2):
                        nc.tensor.matmul(
                            ct_pair[HEAD * h:HEAD * (h + 1), :],
                            v_aug[:, b1, kt_i, HEAD * h:HEAD * (h + 1)],
                            st_prev[:, kt_i, h, :],
                            start=st0, stop=sp0,
                            tile_position=(0, HEAD * h),
                            skip_group_check=True)
                    for h in range(2):
                        nc.tensor.matmul(
                            dn_acc[HEAD * h:HEAD * h + 1, :],
                            ones128[:], st_prev[:, kt_i, h, :],
                            start=st0, stop=sp0,
                            tile_position=(0, HEAD * h),
                            skip_group_check=True)
                # normalize qtile w-2: emitted after kt 0's matmuls so the PE
                # queue has work ahead of the broadcast matmuls while the DVE
                # denominator copies run.
                if 2 <= w and kt_i == 0:
                    q2 = w - 2
                    qs2 = slice(QTW * q2, QTW * (q2 + 1))
                    ct_pair2, dn_acc2 = ct_tiles.pop(q2)
                    dn_sb = dnp.tile([HEAD + 1, QTW], DT, tag="dn")
                    nc.vector.tensor_copy(dn_sb[0:1, :], dn_acc2[0:1, :])
                    nc.vector.tensor_copy(dn_sb[HEAD:HEAD + 1, :],
                                          dn_acc2[HEAD:HEAD + 1, :])
                    rcb_ps = aux.tile([128, QTW], F32, tag="aux")
                    nc.tensor.matmul(rcb_ps[0:HEAD, :], ones_col[0:1, :],
                                     dn_sb[0:1, :], start=True, stop=True,
                                     tile_position=(0, 0))
                    nc.tensor.matmul(rcb_ps[HEAD:128, :],
                                     ones_col[HEAD:HEAD + 1, :],
                                     dn_sb[HEAD:HEAD + 1, :],
                                     start=True, stop=True,
                                     tile_position=(HEAD, HEAD),
                                     skip_group_check=True)
                    # single-pass approximate reciprocal (~18 bits, plenty for
                    # bf16 context) -- the exact multi-pass reciprocal costs
                    # ~3.3us at [128,512] on HW
                    rcb_t = rcbp.tile([128, QTW], F32, tag="rcb")
                    nc.vector.reciprocal_approx_fast(rcb_t[:], rcb_ps[:])
                    ev_t = tmpp.tile([128, QTW], DT, tag="ev")
                    nc.vector.tensor_copy(ev_t[:], ct_pair2[:])
                    nc.vector.tensor_mul(ct_sb[:, qs2], ev_t[:], rcb_t[:])
                # out-projection for qtile w-2, one token tile per 4 kt;
                # PSUM comes from the aux pool so the scores rotation is
                # never blocked behind an out-proj eviction.
                if 2 <= w and kt_i % 4 == 3:
                    t = kt_i // 4
                    tok0 = QTW * (w - 2) + 128 * t
                    lhs = ct_sb[:, tok0:tok0 + 128]
                    o_sb = osbp.tile([128, 2, 512], DT, tag="osb")
                    for half in range(2):
                        o_ps = aux.tile([128, 512], F32, tag="aux")
                        nc.tensor.matmul(o_ps[:], lhs,
                                         wo_sb[:, 512 * half:512 * (half + 1)],
                                         start=True, stop=True)
                        nc.vector.tensor_copy(o_sb[:, half, :], o_ps[:])
                    nc.sync.dma_start(
                        out[tok0:tok0 + 128, :],
                        o_sb[:].rearrange("p a b -> p (a b)"))
            if scoring:
                pass

    nc.compile()
    return nc


def shard_inputs(input, mask, q_w, q_b, k_w, k_b, v_w, v_b, o_w, o_b):
    x = np.asarray(input, np.float32)
    # [128, c, tok] layout: partition p of chunk c is feature row 128c+p
    xt = np.ascontiguousarray(
        x.T.reshape(NKCH, 128, NTOK).transpose(1, 0, 2)).astype(NPDT)
    m = np.asarray(mask, np.float32).reshape(BS, NKT, 128)
    maskd = np.ascontiguousarray(m.transpose(2, 0, 1).reshape(128, BS * NKT))
    scale = 1.0 / math.sqrt(HEAD)

    def warr(w):  # [DIM, LDIM] -> [128, c, LDIM]
        return np.ascontiguousarray(
            w.reshape(NKCH, 128, LDIM).transpose(1, 0, 2)).astype(NPDT)

    in_maps = []
    for c in range(NCORES):
        L = slice(LDIM * c, LDIM * (c + 1))
        in_maps.append({
            "xt": xt,
            "wq": warr(np.ascontiguousarray((q_w[L, :] * scale).T)),
            "wk": warr(np.ascontiguousarray(k_w[L, :].T)),
            "wv": warr(np.ascontiguousarray(v_w[L, :].T)),
            "wo": np.ascontiguousarray(o_w[:, L].T).astype(NPDT),
            "qb": (q_b[L] * scale).astype(np.float32).reshape(LDIM, 1),
            "kb": k_b[L].astype(np.float32).reshape(LDIM, 1),
            "vb": v_b[L].astype(np.float32).reshape(LDIM, 1),
            "maskd": maskd,
        })
    return in_maps


def run(in_maps, **kw):
    if "nc" not in _cache:
        _cache["nc"] = build_program()
    return run_bass_kernel_spmd(_cache["nc"], in_maps,
                                core_ids=list(range(NCORES)), **kw)


def gather_output(results, o_b):
    acc = np.zeros((NTOK, DIM), np.float32)
    for r in results:
        acc += np.asarray(r["out"], np.float32)
    acc += np.asarray(o_b, np.float32)[None, :]
    return acc


def kernel(input, mask, q_w, q_b, k_w, k_b, v_w, v_b, o_w, o_b,
           bs=BS, qlen=QLEN):
    assert int(bs) == BS and int(qlen) == QLEN
    in_maps = shard_inputs(np.asarray(input), np.asarray(mask),
                           np.asarray(q_w), np.asarray(q_b),
                           np.asarray(k_w), np.asarray(k_b),
                           np.asarray(v_w), np.asarray(v_b),
                           np.asarray(o_w), np.asarray(o_b))
    res = run(in_maps)
    return gather_output(res.results, o_b)
